# revision 1
# baseline (speedup 1.0000x reference)
"""CSPN 3x3 propagation on 8 trn2 NeuronCores (batch-parallel).

out[y, x] = sum_{a,c} g[3a+c, y+1, x+1] * src[y+1-a, x+1-c]
  (src = hn zero-padded; the center tap a=c=1 uses h0)

Per core (B=1), row chunks of <=126 output rows. Guide tap planes are
DMA-loaded row-shifted by di=1-a so the DVE product
pt[k] = g_t[u0+k-di] * hn_padded[u0+k] is partition-aligned; the
TensorEngine row-shifts and sums the 9 products in PSUM via exact 0/1
shift matrices (bf16: 1 cycle/row; fp32 would be 4x).

DMA shapes: HWDGE only fans a transfer across the 16 SDMA engine slots
for certain partition counts (measured: 64/96/112/120 spread evenly;
125..128 collapse onto slot 0 at ~1/4 bandwidth). All transfers here are
exact [0:64]/[64:128] halves, one per HWDGE ring. Inputs are host-padded
so every load is a full 128-row window:
  guide  -> flat [3216, 1218]: row 0 zero, then the 9 planes, zero tail
  hn/h0  -> [384, 1216]: row 0 zero, rows 1..352 data, zero tail
  out    -> [384, 1216]: chunk ci stores its 128-row window at ci*128
            (disjoint slots; host reassembles the valid R rows of each)
"""

import numpy as np
import ml_dtypes

import concourse.bacc as bacc
import concourse.mybir as mybir
from concourse import tile
from concourse.bass_utils import run_bass_kernel_spmd

F32 = mybir.dt.float32
BF16 = mybir.dt.bfloat16

B, H, W = 8, 352, 1216
HP, WP = H + 2, W + 2          # padded plane dims (354, 1218)
N_CORES = 8
GROWS = 3216                   # padded flat guide rows (1 + 9*354 + 29)
SROWS = 384                    # padded hn/h0/out rows
CHUNKS = [(1, 126), (127, 126), (253, 100)]   # (i0 = first padded out row, R)
STRIPS = [(0, 512), (512, 512), (1024, 192)]  # (out col w0, width N)


def make_shift_mats():
    """S_d[k, m] = 1 iff k == m + d, d in {0,1,2}; packed [128, 378] bf16."""
    sm = np.zeros((128, 3 * 126), ml_dtypes.bfloat16)
    for d in range(3):
        for m in range(126):
            sm[m + d, d * 126 + m] = 1.0
    return sm


def prep_core_inputs(guide_b: np.ndarray, hn_b: np.ndarray, h0_b: np.ndarray,
                     sm: np.ndarray) -> dict:
    """Pad one sample's inputs to the kernel's DMA-friendly layouts.
    guide_b [9, 354, 1218], hn_b/h0_b [352, 1216] -> dram input dict."""
    gp = np.zeros((GROWS, WP), np.float32)
    gp[1:1 + 9 * HP] = np.asarray(guide_b, np.float32).reshape(9 * HP, WP)
    hp = np.zeros((SROWS, W), np.float32)
    hp[1:1 + H] = np.asarray(hn_b, np.float32)
    h0p = np.zeros((SROWS, W), np.float32)
    h0p[1:1 + H] = np.asarray(h0_b, np.float32)
    return {"guide": gp, "hn": hp, "h0": h0p, "smat": sm}


def assemble_out(out_pad: np.ndarray) -> np.ndarray:
    """[384, 1216] chunk slots -> [352, 1216]."""
    parts = [out_pad[ci * 128:ci * 128 + R] for ci, (_, R) in enumerate(CHUNKS)]
    return np.concatenate(parts, axis=0)


def build(n_iters: int = 1):
    nc = bacc.Bacc()
    g_d = nc.dram_tensor("guide", [GROWS, WP], F32, kind="ExternalInput")
    hn_d = nc.dram_tensor("hn", [SROWS, W], F32, kind="ExternalInput")
    h0_d = nc.dram_tensor("h0", [SROWS, W], F32, kind="ExternalInput")
    sm_d = nc.dram_tensor("smat", [128, 3 * 126], BF16, kind="ExternalInput")
    out_d = nc.dram_tensor("out", [SROWS, W], F32, kind="ExternalOutput")

    with tile.TileContext(nc) as tc:
        with tc.tile_pool(name="const", bufs=1) as cpool, \
             tc.tile_pool(name="gpool", bufs=3) as gpool, \
             tc.tile_pool(name="spool", bufs=2) as spool, \
             tc.tile_pool(name="ppool", bufs=4) as ppool, \
             tc.tile_pool(name="opool", bufs=2) as opool, \
             tc.tile_pool(name="psum", bufs=2, space="PSUM") as pspool:

            smt = cpool.tile([128, 3 * 126], BF16)
            nc.sync.dma_start(out=smt[:], in_=sm_d[:])

            engs = [nc.sync, nc.scalar]

            def dual64(dst, src, flip=0):
                """dst[0:128] <- src, exact 64-row halves, one per ring."""
                engs[flip].dma_start(out=dst[0:64], in_=src[0:64])
                engs[1 - flip].dma_start(out=dst[64:128], in_=src[64:128])

            def chunk_body(ci, i0, R):
                u0 = i0 - 1          # hn tile partition k <-> padded row u0+k
                KL = 128 if R > 100 else 112   # rows per load (112 spreads too)

                # ---- hn/h0 first: every product reads them, so they must
                # lead the per-ring FIFO ahead of the guide stream.
                hnt = spool.tile([128, WP], F32, tag="hn")
                nc.vector.memset(hnt[:, 0:1], 0.0)
                nc.vector.memset(hnt[:, WP - 1:WP], 0.0)
                if KL == 128:
                    engs[0].dma_start(out=hnt[0:64, 1:WP - 1],
                                      in_=hn_d[u0:u0 + 64, :])
                    engs[1].dma_start(out=hnt[64:128, 1:WP - 1],
                                      in_=hn_d[u0 + 64:u0 + 128, :])
                else:
                    engs[0].dma_start(out=hnt[0:KL, 1:WP - 1],
                                      in_=hn_d[u0:u0 + KL, :])

                h0t = spool.tile([128, W], F32, tag="h0")
                if KL == 128:
                    dual64(h0t, h0_d[u0:u0 + 128, :], flip=1)
                else:
                    engs[1].dma_start(out=h0t[0:KL, :], in_=h0_d[u0:u0 + KL, :])

                # ---- guide tiles: partition k of tile t holds g_t row u0+k-di
                gts = []
                for t in range(9):
                    a = t // 3
                    di = 1 - a
                    gt = gpool.tile([128, WP], F32, tag=f"g{t}")
                    base = 1 + t * HP + u0 - di
                    if KL == 128:
                        dual64(gt, g_d[base:base + 128, :], flip=t % 2)
                    else:
                        engs[t % 2].dma_start(out=gt[0:KL, :],
                                              in_=g_d[base:base + KL, :])
                    gts.append(gt)

                # ---- products (bf16) + shift-matmul accumulation
                psts = [pspool.tile([126, 512], F32, tag=f"ps{s}", name=f"ps{s}")
                        for s in range(len(STRIPS))]
                for t in range(9):
                    a, c = t // 3, t % 3
                    d = 2 - a
                    pt = ppool.tile([128, W], BF16, tag="prod")
                    if t == 4:
                        nc.vector.tensor_tensor(pt[0:KL, 0:W],
                                                gts[t][0:KL, 1:1 + W],
                                                h0t[0:KL, 0:W],
                                                mybir.AluOpType.mult)
                    else:
                        nc.vector.tensor_tensor(pt[0:KL, 0:W],
                                                gts[t][0:KL, 1:1 + W],
                                                hnt[0:KL, 2 - c:2 - c + W],
                                                mybir.AluOpType.mult)
                    for s, (w0, N) in enumerate(STRIPS):
                        nc.tensor.matmul(psts[s][0:R, 0:N],
                                         smt[0:KL, d * 126:d * 126 + R],
                                         pt[0:KL, w0:w0 + N],
                                         start=(t == 0), stop=(t == 8))

                # ---- PSUM -> SBUF -> HBM (disjoint 128-row slot per chunk)
                ot = opool.tile([128, W], F32, tag="out")
                nc.gpsimd.memset(ot[96:KL, :], 0.0)
                if KL == 128:
                    for s, (w0, N) in enumerate(STRIPS):
                        nc.scalar.copy(out=ot[0:R, w0:w0 + N],
                                       in_=psts[s][0:R, 0:N])
                    dual64(out_d[ci * 128:ci * 128 + 128, :], ot, flip=ci % 2)
                else:
                    # last chunk: store each strip as soon as its copy lands,
                    # overlapping the remaining matmuls/copies (shorter tail)
                    for s, (w0, N) in enumerate(STRIPS):
                        nc.scalar.copy(out=ot[0:R, w0:w0 + N],
                                       in_=psts[s][0:R, 0:N])
                        engs[s % 2].dma_start(
                            out=out_d[ci * 128:ci * 128 + KL, w0:w0 + N],
                            in_=ot[0:KL, w0:w0 + N])

            def body(_iv=None):
                for ci, (i0, R) in enumerate(CHUNKS):
                    chunk_body(ci, i0, R)

            if n_iters == 1:
                body()
            else:
                with tc.For_i(0, n_iters, 1):
                    body()

    nc.finalize()
    return nc


_nc_cache = {}


def _get_nc(n_iters=1):
    if n_iters not in _nc_cache:
        _nc_cache[n_iters] = build(n_iters)
    return _nc_cache[n_iters]


def kernel(guide_weight: np.ndarray, hn: np.ndarray, h0: np.ndarray) -> np.ndarray:
    """Full inputs: guide_weight [8,9,354,1218], hn/h0 [8,1,352,1216] f32.
    Returns [8,1,352,1216] f32."""
    nc = _get_nc(1)
    sm = make_shift_mats()
    in_maps = [prep_core_inputs(guide_weight[b], hn[b, 0], h0[b, 0], sm)
               for b in range(B)]
    res = run_bass_kernel_spmd(nc, in_maps, list(range(N_CORES)))
    out = np.stack([assemble_out(res.results[b]["out"]) for b in range(B)], axis=0)
    return out[:, None].astype(np.float32)



# revision 2
# speedup vs baseline: 1.0262x; 1.0262x over previous
"""CSPN 3x3 propagation on 8 trn2 NeuronCores (batch-parallel), bf16.

out[y, x] = sum_{i,j} g[3i+j, y+1, x+1] * hn[y+1-i, x+1-j]
  (center tap i=j=1 uses h0; hn/h0 zero-padded outside [0,H)x[0,W))

All wire traffic is bf16 (host casts f32->bf16 in kernel(); output is
stored bf16 and upcast on host — rel err stays ~5e-3 << 2e-2 gate).

No PE: hn is loaded THREE times per chunk with row offsets {0,1,2}
(slot m holds padded row y0+k+m), so every tap's product is already
partition-aligned to the output row. The 9 products + 8-add tree are
split across DVE (2x bf16 mode) and GpSimd; out = acc tile, stored
directly. No PSUM, no shift matrices, no scalar copies.

DMA: one dma_start per 64-row half covers all 9 guide planes via a 3D
access pattern ([rows, 9 planes, 1218] — descriptors of 2436 B), and
one covers all 3 hn shifts. 8 dma_starts per chunk over the 2 HWDGE
queues (sync + scalar). Row counts per transfer stay at 64/96, which
measured as spreading evenly across the 16 SDMA engines.

Layouts (per core, B=1):
  guide -> [3186, 1218] bf16: 9 planes of [354, 1218] concatenated
  hn    -> [354, 1218] bf16: row r = hn[r-1] at cols 1..1216, else 0
  h0    -> [354, 1216] bf16: row r = h0[r-1], rows 0/353 zero
  out   -> [352, 1216] bf16
"""

import numpy as np
import ml_dtypes

import concourse.bacc as bacc
import concourse.mybir as mybir
from concourse import tile
from concourse.ap import AP
from concourse.bass_utils import run_bass_kernel_spmd

BF16 = mybir.dt.bfloat16
MUL = mybir.AluOpType.mult
ADD = mybir.AluOpType.add

B, H, W = 8, 352, 1216
HP, WPAD = H + 2, W + 2        # 354, 1218
GROWS = 9 * HP                 # 3186
N_CORES = 8
CHUNKS = [(0, 128), (128, 128), (256, 96)]   # (y0 = first out row, R)


def prep_core_inputs(guide_b: np.ndarray, hn_b: np.ndarray, h0_b: np.ndarray) -> dict:
    """guide_b [9, 354, 1218] f32, hn_b/h0_b [352, 1216] f32 -> bf16 dram dict."""
    g = np.ascontiguousarray(guide_b, np.float32).astype(ml_dtypes.bfloat16)
    hnp = np.zeros((HP, WPAD), ml_dtypes.bfloat16)
    hnp[1:1 + H, 1:1 + W] = hn_b
    h0p = np.zeros((HP, W), ml_dtypes.bfloat16)
    h0p[1:1 + H, :] = h0_b
    return {"guide": g.reshape(GROWS, WPAD), "hn": hnp, "h0": h0p}


def build():
    nc = bacc.Bacc(enable_partition_id=False)
    g_d = nc.dram_tensor("guide", [GROWS, WPAD], BF16, kind="ExternalInput")
    hn_d = nc.dram_tensor("hn", [HP, WPAD], BF16, kind="ExternalInput")
    h0_d = nc.dram_tensor("h0", [HP, W], BF16, kind="ExternalInput")
    out_d = nc.dram_tensor("out", [H, W], BF16, kind="ExternalOutput")

    with tile.TileContext(nc) as tc:
        with tc.tile_pool(name="gpool", bufs=2) as gpool, \
             tc.tile_pool(name="spool", bufs=2) as spool, \
             tc.tile_pool(name="ppool", bufs=2) as ppool, \
             tc.tile_pool(name="opool", bufs=2) as opool:

            engs = [nc.sync, nc.scalar]

            def chunk_body(ci, y0, R):
                hnt = spool.tile([128, 3, WPAD], BF16, tag="hn", name="hnt")
                h0t = spool.tile([128, W], BF16, tag="h0", name="h0t")
                gt = gpool.tile([128, 9, WPAD], BF16, tag="g", name="gt")

                halves = [(0, 64), (64, 64)] if R == 128 else [(0, 96)]
                for hi, (r0, rows) in enumerate(halves):
                    e = engs[(ci + hi) % 2]
                    e2 = engs[(ci + hi + 1) % 2]
                    # hn: slot m of partition k <- padded row y0+r0+k+m
                    e.dma_start(
                        out=hnt[r0:r0 + rows],
                        in_=AP(hn_d, (y0 + r0) * WPAD,
                               [[WPAD, rows], [WPAD, 3], [1, WPAD]]))
                    e2.dma_start(out=h0t[r0:r0 + rows, :],
                                 in_=h0_d[y0 + 1 + r0:y0 + 1 + r0 + rows, :])
                    # guide: plane t of partition k <- g[t, y0+1+r0+k, :]
                    e.dma_start(
                        out=gt[r0:r0 + rows],
                        in_=AP(g_d, (y0 + 1 + r0) * WPAD,
                               [[WPAD, rows], [HP * WPAD, 9], [1, WPAD]]))

                # products: tap t=3i+j reads gt plane t cols 1..1216 and
                # hn slot m=2-i cols (2-j)..(2-j+1216); center tap uses h0t.
                def gsl(t):
                    return gt[0:R, t, 1:1 + W]

                def hsl(i, j):
                    return hnt[0:R, 2 - i, 2 - j:2 - j + W]

                pts = [ppool.tile([128, W], BF16, tag=f"p{t}", name=f"pt{t}")
                       for t in range(9)]
                # GpSimd computes the center-tap product first, then the add
                # tree's lower levels; DVE does the other 8 products + final
                # combines (2x bf16 mode).
                nc.gpsimd.tensor_tensor(pts[4][0:R], gsl(4), h0t[0:R, :], MUL)
                order = [0, 1, 2, 3, 5, 6, 7, 8]
                for t in order:
                    nc.vector.tensor_tensor(pts[t][0:R], gsl(t), hsl(t // 3, t % 3), MUL)

                s = [ppool.tile([128, W], BF16, tag=f"s{k}", name=f"st{k}")
                     for k in range(4)]
                nc.gpsimd.tensor_tensor(s[0][0:R], pts[0][0:R], pts[1][0:R], ADD)
                nc.gpsimd.tensor_tensor(s[1][0:R], pts[2][0:R], pts[3][0:R], ADD)
                nc.gpsimd.tensor_tensor(s[2][0:R], pts[5][0:R], pts[6][0:R], ADD)
                nc.gpsimd.tensor_tensor(s[3][0:R], pts[7][0:R], pts[8][0:R], ADD)
                t0 = ppool.tile([128, W], BF16, tag="t0", name="t0t")
                t1 = ppool.tile([128, W], BF16, tag="t1", name="t1t")
                nc.gpsimd.tensor_tensor(t0[0:R], s[0][0:R], s[1][0:R], ADD)
                nc.vector.tensor_tensor(t1[0:R], s[2][0:R], s[3][0:R], ADD)
                u = ppool.tile([128, W], BF16, tag="u", name="ut")
                nc.vector.tensor_tensor(u[0:R], t0[0:R], t1[0:R], ADD)
                acc = opool.tile([128, W], BF16, tag="out", name="acct")
                nc.vector.tensor_tensor(acc[0:R], u[0:R], pts[4][0:R], ADD)

                for hi, (r0, rows) in enumerate(halves):
                    engs[(ci + hi) % 2].dma_start(
                        out=out_d[y0 + r0:y0 + r0 + rows, :],
                        in_=acc[r0:r0 + rows, :])

            for ci, (y0, R) in enumerate(CHUNKS):
                chunk_body(ci, y0, R)

    nc.finalize()
    return nc


_nc_cache = {}


def _get_nc():
    if "nc" not in _nc_cache:
        _nc_cache["nc"] = build()
    return _nc_cache["nc"]


def kernel(guide_weight: np.ndarray, hn: np.ndarray, h0: np.ndarray) -> np.ndarray:
    """Full inputs: guide_weight [8,9,354,1218], hn/h0 [8,1,352,1216] f32.
    Returns [8,1,352,1216] f32."""
    nc = _get_nc()
    in_maps = [prep_core_inputs(guide_weight[b], hn[b, 0], h0[b, 0])
               for b in range(B)]
    res = run_bass_kernel_spmd(nc, in_maps, list(range(N_CORES)))
    out = np.stack([np.asarray(res.results[b]["out"]) for b in range(B)], axis=0)
    return out[:, None].astype(np.float32)


# revision 5
# speedup vs baseline: 1.3817x; 1.3464x over previous
"""CSPN 3x3 propagation on 8 trn2 NeuronCores (batch-parallel), bf16.

out[y, x] = sum_{i,j} g[3i+j, y+1, x+1] * hn[y+1-i, x+1-j]
  (center tap i=j=1 uses h0; hn/h0 zero-padded outside [0,H)x[0,W))

All wire traffic is bf16 (host casts f32->bf16 inside kernel(); output
is stored bf16 and upcast on host — rel err stays well under the 2e-2
gate). ~12 MB per core vs 21.6 MB for the f32 version.

Structure (per chunk of <=126 output rows):
- DVE computes the 9 tap products in 2x bf16 perf mode (~0.85us each).
  2x requires 4-byte-aligned element offsets on EVERY operand (measured:
  odd offsets drop to 1x, and concurrent GpSimd tensor_tensor poisons
  DVE to 0.25x — so GpSimd does no compute). Alignment by layout:
    guide planes pre-sliced to cols 1..1216 (slices start at 0),
    hn [*,1218] col-padded serves j=0/j=2 (starts 2/0),
    hn2 [*,1216] unpadded serves j=1 (start 0), h0 likewise.
- Guide tile for tap t=3i+j is DMA-loaded row-shifted (first row
  y0+i-1), so the DVE product pt[k] = g_t[..] * hn[y0+k ..] is
  partition-aligned; the PE row-shifts by d=2-i and sums all 9 products
  into PSUM via exact 0/1 shift matrices (bf16, 1 col/cycle).
- Scalar engine copies PSUM f32 -> SBUF bf16 (3 x 512/512/192 strips),
  then the result is stored.

DMA: per chunk, one dma_start per 64/112-row half covers a whole
3-plane guide band via a 3D access pattern (2432 B descriptors spread
across all 16 SDMA engines); hn/hn2/h0 are plain 2D loads.

Layouts (per core, B=1):
  guide -> [3200, 1216] bf16: zero row, then 9 planes of [354, 1216]
           (orig cols 1..1217), zero tail
  hn    -> [368, 1218] bf16: row r = hn[r-1] at cols 1..1216, else 0
  hn2   -> [368, 1216] bf16: row r = hn[r-1], rows 0/353.. zero
  h0    -> [368, 1216] bf16: row r = h0[r-1]
  out   -> [352, 1216] bf16
"""

import numpy as np
import ml_dtypes

import concourse.bacc as bacc
import concourse.mybir as mybir
from concourse import tile
from concourse.ap import AP
from concourse.bass_utils import run_bass_kernel_spmd

BF16 = mybir.dt.bfloat16
F32 = mybir.dt.float32
MUL = mybir.AluOpType.mult

B, H, W = 8, 352, 1216
HP, WPAD = H + 2, W + 2        # 354, 1218
GROWS = 1 + 9 * HP + 13        # 3200
SROWS = 368
N_CORES = 8
CHUNKS = [(0, 126, 128), (126, 126, 128), (252, 100, 112)]  # (y0, R, KL)
STRIPS = [(0, 512), (512, 512), (1024, 192)]


def make_shift_mats():
    """S_d[k, m] = 1 iff k == m + d, d in {0,1,2}; packed [128, 378] bf16."""
    sm = np.zeros((128, 3 * 126), ml_dtypes.bfloat16)
    for d in range(3):
        for m in range(126):
            sm[m + d, d * 126 + m] = 1.0
    return sm


def prep_core_inputs(guide_b: np.ndarray, hn_b: np.ndarray, h0_b: np.ndarray,
                     sm: np.ndarray) -> dict:
    """guide_b [9, 354, 1218] f32, hn_b/h0_b [352, 1216] f32 -> bf16 dram dict."""
    gp = np.zeros((GROWS, W), ml_dtypes.bfloat16)
    gp[1:1 + 9 * HP] = np.asarray(guide_b, np.float32)[:, :, 1:1 + W].reshape(9 * HP, W)
    hnp = np.zeros((SROWS, WPAD), ml_dtypes.bfloat16)
    hnp[1:1 + H, 1:1 + W] = hn_b
    hn2 = np.zeros((SROWS, W), ml_dtypes.bfloat16)
    hn2[1:1 + H, :] = hn_b
    h0p = np.zeros((SROWS, W), ml_dtypes.bfloat16)
    h0p[1:1 + H, :] = h0_b
    return {"guide": gp, "hn": hnp, "hn2": hn2, "h0": h0p, "smat": sm}


def build():
    nc = bacc.Bacc(enable_partition_id=False)
    g_d = nc.dram_tensor("guide", [GROWS, W], BF16, kind="ExternalInput")
    hn_d = nc.dram_tensor("hn", [SROWS, WPAD], BF16, kind="ExternalInput")
    hn2_d = nc.dram_tensor("hn2", [SROWS, W], BF16, kind="ExternalInput")
    h0_d = nc.dram_tensor("h0", [SROWS, W], BF16, kind="ExternalInput")
    sm_d = nc.dram_tensor("smat", [128, 3 * 126], BF16, kind="ExternalInput")
    out_d = nc.dram_tensor("out", [H, W], BF16, kind="ExternalOutput")

    with tile.TileContext(nc) as tc:
        with tc.tile_pool(name="const", bufs=1) as cpool, \
             tc.tile_pool(name="gpool", bufs=2) as gpool, \
             tc.tile_pool(name="spool", bufs=2) as spool, \
             tc.tile_pool(name="ppool", bufs=2) as ppool, \
             tc.tile_pool(name="opool", bufs=2) as opool, \
             tc.tile_pool(name="psum", bufs=2, space="PSUM") as pspool:

            smt = cpool.tile([128, 3 * 126], BF16)
            nc.sync.dma_start(out=smt[0:64, :], in_=sm_d[0:64, :])
            nc.scalar.dma_start(out=smt[64:128, :], in_=sm_d[64:128, :])

            engs = [nc.sync, nc.scalar]

            def chunk_body(ci, y0, R, KL):
                hnt = spool.tile([128, WPAD], BF16, tag="hn", name="hnt")
                hn2t = spool.tile([128, W], BF16, tag="hn2", name="hn2t")
                h0t = spool.tile([128, W], BF16, tag="h0", name="h0t")
                gt = gpool.tile([128, 9, W], BF16, tag="g", name="gt")

                halves = [(0, 64), (64, 64)] if KL == 128 else [(0, 112)]
                for hi, (r0, rows) in enumerate(halves):
                    e = engs[(ci + hi) % 2]
                    e2 = engs[(ci + hi + 1) % 2]
                    e.dma_start(out=hnt[r0:r0 + rows, :],
                                in_=hn_d[y0 + r0:y0 + r0 + rows, :])
                    e2.dma_start(out=hn2t[r0:r0 + rows, :],
                                 in_=hn2_d[y0 + r0:y0 + r0 + rows, :])
                    e.dma_start(out=h0t[r0:r0 + rows, :],
                                in_=h0_d[y0 + r0:y0 + r0 + rows, :])
                    # guide band a: planes 3a..3a+2, tile row k <- flat row
                    # 1 + (3a+p)*HP + y0 + a - 1 + k
                    for a in range(3):
                        base = 1 + 3 * a * HP + y0 + a - 1 + r0
                        engs[(ci + hi + a) % 2].dma_start(
                            out=gt[r0:r0 + rows, 3 * a:3 * a + 3, :],
                            in_=AP(g_d, base * W,
                                   [[W, rows], [HP * W, 3], [1, W]]))

                # products (DVE 2x) + PE shift-matmul accumulation
                psts = [pspool.tile([126, 512], F32, tag=f"ps{s}", name=f"ps{s}")
                        for s in range(len(STRIPS))]
                pt = ppool.tile([128, 9, W], BF16, tag="p", name="pt")
                for t in range(9):
                    i, j = t // 3, t % 3
                    d = 2 - i
                    if t == 4:
                        src = h0t[0:KL, :]
                    elif j == 1:
                        src = hn2t[0:KL, :]
                    else:
                        src = hnt[0:KL, 2 - j:2 - j + W]
                    nc.vector.tensor_tensor(pt[0:KL, t], gt[0:KL, t], src, MUL)
                    for s, (w0, N) in enumerate(STRIPS):
                        nc.tensor.matmul(psts[s][0:R, 0:N],
                                         smt[0:KL, d * 126:d * 126 + R],
                                         pt[0:KL, t, w0:w0 + N],
                                         start=(t == 0), stop=(t == 8))

                # PSUM f32 -> SBUF bf16 -> HBM
                ot = opool.tile([128, W], BF16, tag="out", name="ot")
                for s, (w0, N) in enumerate(STRIPS):
                    nc.scalar.copy(out=ot[0:R, w0:w0 + N], in_=psts[s][0:R, 0:N])
                stores = [(0, 64), (64, R - 64)] if R == 126 else [(0, R)]
                for si, (r0, rows) in enumerate(stores):
                    engs[(ci + si) % 2].dma_start(
                        out=out_d[y0 + r0:y0 + r0 + rows, :],
                        in_=ot[r0:r0 + rows, :])

            for ci, (y0, R, KL) in enumerate(CHUNKS):
                chunk_body(ci, y0, R, KL)

    nc.finalize()
    return nc


_nc_cache = {}


def _get_nc():
    if "nc" not in _nc_cache:
        _nc_cache["nc"] = build()
    return _nc_cache["nc"]


def kernel(guide_weight: np.ndarray, hn: np.ndarray, h0: np.ndarray) -> np.ndarray:
    """Full inputs: guide_weight [8,9,354,1218], hn/h0 [8,1,352,1216] f32.
    Returns [8,1,352,1216] f32."""
    nc = _get_nc()
    sm = make_shift_mats()
    in_maps = [prep_core_inputs(guide_weight[b], hn[b, 0], h0[b, 0], sm)
               for b in range(B)]
    res = run_bass_kernel_spmd(nc, in_maps, list(range(N_CORES)))
    out = np.stack([np.asarray(res.results[b]["out"]) for b in range(B)], axis=0)
    return out[:, None].astype(np.float32)


# revision 6
# speedup vs baseline: 1.4855x; 1.0751x over previous
"""CSPN 3x3 propagation on 8 trn2 NeuronCores (batch-parallel), bf16.

out[y, x] = sum_{i,j} g[3i+j, y+1, x+1] * hn[y+1-i, x+1-j]
  (center tap i=j=1 uses h0; hn/h0 zero-padded outside [0,H)x[0,W))

All wire traffic is bf16 (host casts f32->bf16 inside kernel(); output
is stored bf16 and upcast on host — rel err stays well under the 2e-2
gate). ~11.5 MB per core vs 21.6 MB for the f32 version.

Structure (per chunk of <=126 output rows):
- DVE computes the 9 tap products in 2x bf16 perf mode (~0.9us each).
  2x requires 4-byte-aligned element offsets on EVERY operand (measured:
  odd offsets drop to 1x; concurrent GpSimd tensor_tensor poisons DVE to
  0.25x — so GpSimd does no compute). Alignment by layout:
    guide planes pre-sliced to cols 1..1216 (slices start at 0),
    hn [*,1218] col-padded serves j=0/j=2 (starts 2/0),
    hn2/h0 [*,2,1216] unpadded interleaved serve j=1 (start 0).
- Taps within a band i share the PE shift d=2-i, so DVE also pre-sums
  tap pairs (q[a] = p_{3a} + p_{3a+1}) with ONE wide strided add; the PE
  then shift-sums only 6 streams (p2,p5,p8 early + q[0..2]) into PSUM
  via exact 0/1 shift matrices instead of 9.
- Scalar engine copies PSUM f32 -> SBUF bf16 (512/512/192 strips).

Guide tile for tap t=3i+j is DMA-loaded row-shifted (first row y0+i-1),
so products are partition-aligned to hn rows. One dma_start per
64/112-row half covers a whole 3-plane guide band via a 3D access
pattern (2432 B descriptors spread across all 16 SDMA engines).

Layouts (per core, B=1):
  guide -> [3200, 1216] bf16: zero row, then 9 planes of [354, 1216]
           (orig cols 1..1217), zero tail
  hn    -> [368, 1218] bf16: row r = hn[r-1] at cols 1..1216, else 0
  hh    -> [368, 2, 1216] bf16: row r = (hn[r-1], h0[r-1]), pad rows 0
  out   -> [352, 1216] bf16
"""

import numpy as np
import ml_dtypes

import concourse.bacc as bacc
import concourse.mybir as mybir
from concourse import tile
from concourse.ap import AP
from concourse.bass_utils import run_bass_kernel_spmd

BF16 = mybir.dt.bfloat16
F32 = mybir.dt.float32
MUL = mybir.AluOpType.mult
ADD = mybir.AluOpType.add

B, H, W = 8, 352, 1216
HP, WPAD = H + 2, W + 2        # 354, 1218
GROWS = 1 + 9 * HP + 13        # 3200
SROWS = 368
N_CORES = 8
CHUNKS = [(0, 126, 128), (126, 126, 128), (252, 100, 112)]  # (y0, R, KL)
STRIPS = [(0, 512), (512, 512), (1024, 192)]


def make_shift_mats():
    """S_d[k, m] = 1 iff k == m + d, d in {0,1,2}; packed [128, 378] bf16."""
    sm = np.zeros((128, 3 * 126), ml_dtypes.bfloat16)
    for d in range(3):
        for m in range(126):
            sm[m + d, d * 126 + m] = 1.0
    return sm


def prep_core_inputs(guide_b: np.ndarray, hn_b: np.ndarray, h0_b: np.ndarray,
                     sm: np.ndarray) -> dict:
    """guide_b [9, 354, 1218] f32, hn_b/h0_b [352, 1216] f32 -> bf16 dram dict."""
    gp = np.zeros((GROWS, W), ml_dtypes.bfloat16)
    gp[1:1 + 9 * HP] = np.asarray(guide_b, np.float32)[:, :, 1:1 + W].reshape(9 * HP, W)
    hnp = np.zeros((SROWS, WPAD), ml_dtypes.bfloat16)
    hnp[1:1 + H, 1:1 + W] = hn_b
    hh = np.zeros((SROWS, 2, W), ml_dtypes.bfloat16)
    hh[1:1 + H, 0, :] = hn_b
    hh[1:1 + H, 1, :] = h0_b
    return {"guide": gp, "hn": hnp, "hh": hh.reshape(2 * SROWS, W), "smat": sm}


def build():
    nc = bacc.Bacc(enable_partition_id=False)
    g_d = nc.dram_tensor("guide", [GROWS, W], BF16, kind="ExternalInput")
    hn_d = nc.dram_tensor("hn", [SROWS, WPAD], BF16, kind="ExternalInput")
    hh_d = nc.dram_tensor("hh", [2 * SROWS, W], BF16, kind="ExternalInput")
    sm_d = nc.dram_tensor("smat", [128, 3 * 126], BF16, kind="ExternalInput")
    out_d = nc.dram_tensor("out", [H, W], BF16, kind="ExternalOutput")

    with tile.TileContext(nc) as tc:
        with tc.tile_pool(name="const", bufs=1) as cpool, \
             tc.tile_pool(name="gpool", bufs=3) as gpool, \
             tc.tile_pool(name="spool", bufs=3) as spool, \
             tc.tile_pool(name="ppool", bufs=2) as ppool, \
             tc.tile_pool(name="opool", bufs=2) as opool, \
             tc.tile_pool(name="psum", bufs=2, space="PSUM") as pspool:

            smt = cpool.tile([128, 3 * 126], BF16)
            nc.sync.dma_start(out=smt[0:64, :], in_=sm_d[0:64, :])
            nc.scalar.dma_start(out=smt[64:128, :], in_=sm_d[64:128, :])

            engs = [nc.sync, nc.scalar]

            def chunk_body(ci, y0, R, KL):
                hnt = spool.tile([128, WPAD], BF16, tag="hn", name="hnt")
                hht = spool.tile([128, 2, W], BF16, tag="hh", name="hht")
                gt = gpool.tile([128, 9, W], BF16, tag="g", name="gt")

                halves = [(0, 64), (64, 64)] if KL == 128 else [(0, 112)]
                for hi, (r0, rows) in enumerate(halves):
                    e = engs[(ci + hi) % 2]
                    e2 = engs[(ci + hi + 1) % 2]
                    e.dma_start(out=hnt[r0:r0 + rows, :],
                                in_=hn_d[y0 + r0:y0 + r0 + rows, :])
                    e2.dma_start(out=hht[r0:r0 + rows],
                                 in_=hh_d[2 * (y0 + r0):2 * (y0 + r0 + rows), :])
                    # guide band a: planes 3a..3a+2, tile row k <- flat row
                    # 1 + (3a+p)*HP + y0 + a - 1 + k
                    for a in range(3):
                        base = 1 + 3 * a * HP + y0 + a - 1 + r0
                        engs[(ci + hi + a) % 2].dma_start(
                            out=gt[r0:r0 + rows, 3 * a:3 * a + 3, :],
                            in_=AP(g_d, base * W,
                                   [[W, rows], [HP * W, 3], [1, W]]))

                def src_for(t):
                    i, j = t // 3, t % 3
                    if t == 4:
                        return hht[0:KL, 1, :]
                    if j == 1:
                        return hht[0:KL, 0, :]
                    return hnt[0:KL, 2 - j:2 - j + W]

                # DVE: band-leftover products first (PE starts early), then
                # the 6 paired products and ONE wide strided tri-add.
                pt = ppool.tile([128, 9, W], BF16, tag="p", name="pt")
                for t in (2, 5, 8, 0, 3, 6, 1, 4, 7):
                    nc.vector.tensor_tensor(pt[0:KL, t], gt[0:KL, t], src_for(t), MUL)
                q = ppool.tile([128, 3, W], BF16, tag="q", name="qt")
                nc.vector.tensor_tensor(
                    q[0:KL],
                    AP(pt.tensor, 0, [[9 * W, KL], [3 * W, 3], [1, W]]),
                    AP(pt.tensor, W, [[9 * W, KL], [3 * W, 3], [1, W]]),
                    ADD)

                # PE: 6 shift-matmul streams per strip (d = 2 - band)
                psts = [pspool.tile([126, 512], F32, tag=f"ps{s}", name=f"ps{s}")
                        for s in range(len(STRIPS))]
                for s, (w0, N) in enumerate(STRIPS):
                    for mi, (mv, d) in enumerate([(pt[0:KL, 2, w0:w0 + N], 2),
                                                  (pt[0:KL, 5, w0:w0 + N], 1),
                                                  (pt[0:KL, 8, w0:w0 + N], 0),
                                                  (q[0:KL, 0, w0:w0 + N], 2),
                                                  (q[0:KL, 1, w0:w0 + N], 1),
                                                  (q[0:KL, 2, w0:w0 + N], 0)]):
                        nc.tensor.matmul(psts[s][0:R, 0:N],
                                         smt[0:KL, d * 126:d * 126 + R], mv,
                                         start=(mi == 0), stop=(mi == 5))

                # PSUM f32 -> SBUF bf16 -> HBM
                ot = opool.tile([128, W], BF16, tag="out", name="ot")
                for s, (w0, N) in enumerate(STRIPS):
                    nc.scalar.copy(out=ot[0:R, w0:w0 + N], in_=psts[s][0:R, 0:N])
                stores = [(0, 64), (64, R - 64)] if R == 126 else [(0, R)]
                for si, (r0, rows) in enumerate(stores):
                    engs[(ci + si) % 2].dma_start(
                        out=out_d[y0 + r0:y0 + r0 + rows, :],
                        in_=ot[r0:r0 + rows, :])

            for ci, (y0, R, KL) in enumerate(CHUNKS):
                chunk_body(ci, y0, R, KL)

    nc.finalize()
    return nc


_nc_cache = {}


def _get_nc():
    if "nc" not in _nc_cache:
        _nc_cache["nc"] = build()
    return _nc_cache["nc"]


def kernel(guide_weight: np.ndarray, hn: np.ndarray, h0: np.ndarray) -> np.ndarray:
    """Full inputs: guide_weight [8,9,354,1218], hn/h0 [8,1,352,1216] f32.
    Returns [8,1,352,1216] f32."""
    nc = _get_nc()
    sm = make_shift_mats()
    in_maps = [prep_core_inputs(guide_weight[b], hn[b, 0], h0[b, 0], sm)
               for b in range(B)]
    res = run_bass_kernel_spmd(nc, in_maps, list(range(N_CORES)))
    out = np.stack([np.asarray(res.results[b]["out"]) for b in range(B)], axis=0)
    return out[:, None].astype(np.float32)
